# revision 36
# baseline (speedup 1.0000x reference)
"""Trainium2 Bass kernel for CrossNonLocalBlock — v4 (engine-rebalanced).

Shapes (hardcoded): B=8, Cs=Ct=256, Ci=128, H=W=64 (N=4096 spatial).
Sharding: data-parallel over batch (1 batch element per NeuronCore, 8 cores);
params replicated; BN batch statistics all-reduced in-kernel.

Per-core structure:
  phase 0: one packed param DMA + 8 big x/l DMAs (l stays resident for the
           phase-3 residual); phi fully + theta c0 + g groups 0/1 up front,
           remaining theta chunks / g groups trail into early nt iterations
           through the S-ring PSUM slots.
  phase 1: attention n-loop.  Per nt: 8 S matmuls (512-col) into a 2-slot
           PSUM ring; DVE "schraudolph-16" exp on cols 0:2048 (2 ops,
           int16 bits == bf16(exp)); ACT exp on cols 2048:4096 (2 ops,
           fused row-sum accum); Pool folds the sch halves, DVE
           tensor_tensor_reduce finishes the row-sum, Pool combines z and
           scales g by 1/z; Y matmuls for cols 0:2048 accumulate in PSUM
           at lag 2.
  phase 2: window-major Y matmuls for cols 2048:4096 from stored f, with
           y0/y1 drains and W-conv statistics chunks (ACT accum for S1,
           fused DVE square+reduce for S2) pipelined inside.
  phase 3: 2KB AllReduce of BN stats with W-conv recompute prefired under
           it; BN normalize + residual (resident l tiles) + bf16 stores.

Pool lifetimes use per-side allocation stacks (SBUF/PSUM each have left and
right stacks) plus ExitStacks so psY0 (right) can close mid-phase-2 while
psY1 (left) stays live, and the sch-bits ring (SBUF right) survives into
phase 2 for the last two Y matmul groups.
"""

import sys
from contextlib import ExitStack

import numpy as np
import ml_dtypes

if "/opt/trn_rl_repo" not in sys.path:
    sys.path.insert(0, "/opt/trn_rl_repo")

B, CS, CT, CI, N = 8, 256, 256, 128, 4096
NT = N // 128          # 32 n-tiles
M0 = 2048              # PSUM-resident Y columns (sch16 cols)
M1 = N - M0            # stored-f columns (ACT exp cols)
BN_EPS = 1e-5
N_CORES = 8

# schraudolph-16: bf16bits(exp(x)) ~= int16(x * 128/ln2 + (127*128 - 5.5)).
# B is recentered (+249.5) so the measured logit range [-84.4, 80.6] sits
# mid-window (int16 bits must stay in (0, 32767): S in (-89.3, 88.1)).  That
# scales f by 2^(249.5/128); the ACT-exp chunks get the matching ln-space
# bias so row-sums stay consistent (softmax cancels the common factor).
SCH_A = 128.0 / float(np.log(2.0))
SCH_B = 16500.0
ACT_BIAS = (SCH_B - (127.0 * 128.0 - 5.5)) * float(np.log(2.0)) / 128.0

_CACHE = {}
ROLES = {}


def _tag(inst, role):
    try:
        ROLES[inst.ins.name] = role
    except Exception:
        pass
    return inst


def _build(n_cores: int, no_collective: bool = False):
    import concourse.mybir as mybir
    import concourse.tile as tile
    from concourse import bacc

    f32 = mybir.dt.float32
    bf16 = mybir.dt.bfloat16
    i16 = mybir.dt.int16
    AF = mybir.ActivationFunctionType
    AX = mybir.AxisListType
    ALU = mybir.AluOpType

    nc = bacc.Bacc("TRN2", target_bir_lowering=False, debug=False,
                   num_devices=n_cores)

    # ---- DRAM I/O (per-core) ----
    # pwb packed bf16 [128, 1024]: thw k0|k1 | phw k0|k1 | gw k0|k1 | wwT
    pwb_d = nc.dram_tensor("pwb", [128, 1024], bf16, kind="ExternalInput").ap()
    # pf: packed f32 [128, 10]: thb|phb|wb0|wb1|gam0|gam1|bet0|bet1|-|-
    pf_d = nc.dram_tensor("pf", [128, 10], f32, kind="ExternalInput").ap()
    gbr_d = nc.dram_tensor("g_b_bf", [1, CI], bf16, kind="ExternalInput").ap()
    x_d = nc.dram_tensor("x", [CS, N], bf16, kind="ExternalInput").ap()
    l_d = nc.dram_tensor("lres", [CT, N], bf16, kind="ExternalInput").ap()
    out = nc.dram_tensor("out", [CT, N], bf16, kind="ExternalOutput").ap()

    with tile.TileContext(nc) as tc, \
         tc.tile_pool(name="persist", bufs=1) as pp, \
         tc.tile_pool(name="dram", bufs=1, space="DRAM") as dp, \
         tc.tile_pool(name="fstore", bufs=1) as fsp:
        gts = pp.tile([128, NT * CI], bf16)  # gT tiles (scaled in-loop) 8KB
        pwb = pp.tile([128, 1024], bf16)     # 2KB
        pf = pp.tile([128, 10], f32)
        gbr = pp.tile([1, CI], bf16)
        onesb = pp.tile([1, 128], bf16)
        stats = pp.tile([128, 4], f32)
        statsg = pp.tile([128, 4], f32)
        s1c = pp.tile([128, 8], f32)
        s2c = pp.tile([128, 8], f32)
        actb = pp.tile([128, 1], f32)
        epsb = pp.tile([128, 1], f32)
        # l resident: [k half][col half 2048] — reused for the residual
        lt = [[pp.tile([128, 2048], bf16, name=f"lt{k}{h}")
               for h in range(2)] for k in range(2)]

        thw = pwb[:, 0:2 * CI]
        phw = pwb[:, 2 * CI:4 * CI]
        gwb = pwb[:, 4 * CI:6 * CI]
        wwb = pwb[:, 6 * CI:8 * CI]
        thb = pf[:, 0:1]
        phb = pf[:, 1:2]

        cc_in = dp.tile([128, 4], f32)
        cc_out = dp.tile([128, 4], f32,
                         addr_space="Shared" if n_cores > 1 else "Local")

        fst = [fsp.tile([128, 8 * M1], bf16, name=f"fst{i}")
               for i in range(4)]

        def fdst(nt):
            return fst[nt // 8], (nt % 8) * M1

        # x/theta/phi live from phase 0 through the end of the n-loop
        es01 = ExitStack()
        p01 = es01.enter_context(tc.tile_pool(name="p01", bufs=1))
        theta = p01.tile([CI, N], bf16)      # 8KB/part
        phi = p01.tile([CI, N], bf16)        # 8KB/part
        xt = [[p01.tile([128, 2048], bf16, name=f"x{k}{h}")
               for h in range(2)] for k in range(2)]

        nc.vector.memset(onesb[:, :], 1.0)
        nc.vector.memset(actb[:, :], ACT_BIAS)
        nc.vector.memset(epsb[:, :], BN_EPS)

        # ---- DMAs: params first (biases gate every projection drain),
        #      x/l spread across 4 engine queues so HWDGE gen and the
        #      transfers run in parallel
        nc.sync.dma_start(pwb[:, :], pwb_d[:, :])
        nc.scalar.dma_start(pf[:, :], pf_d[:, :])
        nc.scalar.dma_start(gbr[:, :], gbr_d[:, :])
        nc.scalar.dma_start(xt[0][0][:, :], x_d[0:128, 0:2048])
        nc.sync.dma_start(xt[1][0][:, :], x_d[128:256, 0:2048])
        nc.sync.dma_start(lt[0][0][:, :], l_d[0:128, 0:2048])
        nc.scalar.dma_start(lt[1][0][:, :], l_d[128:256, 0:2048])
        nc.sync.dma_start(lt[0][1][:, :], l_d[0:128, 2048:N])
        nc.scalar.dma_start(lt[1][1][:, :], l_d[128:256, 2048:N])
        # x h1 feeds only the trailing projection units (nt >= 8)
        nc.scalar.dma_start(xt[0][1][:, :], x_d[0:128, 2048:N])
        nc.sync.dma_start(xt[1][1][:, :], x_d[128:256, 2048:N])

        # preload the exp/identity activation table during the DMA wait
        with tc.tile_pool(name="dum", bufs=1) as dum:
            tdum = dum.tile([128, 1], f32)
            nc.scalar.activation(tdum[:, :], actb[:, :], AF.Exp, scale=1.0)

        # ======== projection helpers (PSUM tile passed in) ========
        def theta_half(c2, pt, on_act):
            # 512-col theta slab c2 in 0..7
            csl = slice(c2 * 512, (c2 + 1) * 512)
            xs = slice((c2 % 4) * 512, (c2 % 4 + 1) * 512)
            for k in range(2):
                nc.tensor.matmul(
                    pt[:, :], thw[:, k * CI:(k + 1) * CI],
                    xt[k][c2 // 4][:, xs],
                    start=(k == 0), stop=(k == 1))
            if on_act:
                nc.scalar.activation(theta[:, csl], pt[:, :],
                                     AF.Identity, bias=thb, scale=1.0)
            else:
                nc.vector.tensor_scalar(theta[:, csl], pt[:, :],
                                        thb, None, ALU.add)

        def phi_chunk(c, pl, on_act):
            csl = slice(c * 1024, (c + 1) * 1024)
            loc = (c % 2) * 1024
            for h in range(2):
                hs = slice(h * 512, (h + 1) * 512)
                ls = slice(loc + h * 512, loc + (h + 1) * 512)
                for k in range(2):
                    nc.tensor.matmul(
                        pl[:, hs], phw[:, k * CI:(k + 1) * CI],
                        lt[k][c // 2][:, ls],
                        start=(k == 0), stop=(k == 1))
            if on_act:
                nc.scalar.activation(phi[:, csl], pl[:, :],
                                     AF.Identity, bias=phb, scale=1.0)
            else:
                nc.vector.tensor_scalar(phi[:, csl], pl[:, :],
                                        phb, None, ALU.add)

        def gt_group(grp, pg, on_act):
            # 4 nt tiles -> one [128,512] PSUM -> gts (bf16)
            h = grp // 4
            for j in range(4):
                nt = grp * 4 + j
                lo = (nt % 16) * 128
                jsl = slice(j * 128, (j + 1) * 128)
                nc.tensor.matmul(pg[:, jsl], xt[0][h][:, lo:lo + 128],
                                 gwb[:, 0:CI], start=True, stop=False)
                nc.tensor.matmul(pg[:, jsl], xt[1][h][:, lo:lo + 128],
                                 gwb[:, CI:2 * CI], start=False, stop=False)
                nc.tensor.matmul(pg[:, jsl], onesb[:, :], gbr[:, :],
                                 start=False, stop=True)
            gsl = slice(grp * 512, (grp + 1) * 512)
            if on_act:
                nc.scalar.activation(gts[:, gsl], pg[:, :],
                                     AF.Identity, scale=1.0)
            else:
                nc.vector.tensor_copy(gts[:, gsl], pg[:, :])

        # ============ phase 0 (up-front part) ============
        with tc.tile_pool(name="ps0", bufs=2, space="PSUM") as ps0, \
             tc.tile_pool(name="psg", bufs=2, space="PSUM") as psg:
            pp0 = ps0.tile([128, 1024], f32, tag="p0")
            phi_chunk(0, pp0, False)
            pp1 = ps0.tile([128, 1024], f32, tag="p0")
            phi_chunk(1, pp1, True)
            pt0 = psg.tile([128, 512], f32, tag="pg")
            theta_half(0, pt0, True)
            pt1 = psg.tile([128, 512], f32, tag="pg")
            theta_half(1, pt1, False)
            pg0 = psg.tile([128, 512], f32, tag="pg")
            gt_group(0, pg0, False)
            pp2 = ps0.tile([128, 1024], f32, tag="p0")
            phi_chunk(2, pp2, False)
            pg1 = psg.tile([128, 512], f32, tag="pg")
            gt_group(1, pg1, True)
            pp3 = ps0.tile([128, 1024], f32, tag="p0")
            phi_chunk(3, pp3, True)

        # ============ phase 1: attention n-loop ============
        es_y0 = ExitStack()
        psY0 = es_y0.enter_context(
            tc.tile_pool(name="psY0", bufs=1, space="PSUM", side="right"))
        y0 = psY0.tile([CI, M0], f32)

        # sch-bits ring + small per-nt tiles: SBUF right stack so they can
        # outlive p01 (closed right after the loop) into early phase 2
        es_bb = ExitStack()
        bbp = es_bb.enter_context(
            tc.tile_pool(name="bbp", bufs=3, side="right"))
        scp = es_bb.enter_context(
            tc.tile_pool(name="scp", bufs=3, side="right"))
        jkp = es_bb.enter_context(
            tc.tile_pool(name="jkp", bufs=1, side="right"))
        zcp = es_bb.enter_context(
            tc.tile_pool(name="zcp", bufs=3, side="right"))
        zp = es_bb.enter_context(
            tc.tile_pool(name="zp", bufs=3, side="right"))

        st = {}
        chain_st = {}
        fold_st = {}

        def emit_chain(j):
            # full z-chain for nt j on DVE: fold(j) completed >=1 full
            # iteration ago, ACT accums too
            scr_p, zc_p, bv_p = fold_st.pop(j)
            jk = jkp.tile([128, 512], bf16, tag="jk")
            _tag(nc.vector.tensor_tensor(
                jk[:, :], scr_p[:, 0:512], scr_p[:, 512:1024], ALU.add),
                 f"ttf.{j}")
            _tag(nc.vector.reduce_sum(zc_p[:, 0:1], jk[:, :], axis=AX.X),
                 f"ttr.{j}")
            z2 = zp.tile([128, 1], f32, tag="z2")
            _tag(nc.vector.reduce_sum(z2[:, :], zc_p[:, 0:5], axis=AX.X),
                 f"z.{j}")
            rz = zp.tile([128, 1], f32, tag="rz")
            nc.vector.reciprocal(rz[:, :], z2[:, :])
            g_p = gts[:, j * CI:(j + 1) * CI]
            _tag(nc.vector.tensor_scalar(
                g_p, g_p, rz[:, :], None, ALU.mult), f"gdiv.{j}")

        def emit_Y(j):
            bb = st.pop(j)
            bv = bb.bitcast(bf16)
            g_j = gts[:, j * CI:(j + 1) * CI]
            for w in range(4):
                _tag(nc.tensor.matmul(
                    y0[:, w * 512:(w + 1) * 512],
                    g_j, bv[:, w * 512:(w + 1) * 512],
                    start=(j == 0), stop=(j == NT - 1)), f"Ymm.{j}.{w}")

        with tc.tile_pool(name="psS", bufs=4, space="PSUM") as psS:
            # trailing phase-0 units: emitted inside early nt iterations
            # through the S-ring slots
            def trailing(slot):
                kind, arg = slot
                if kind == "th":
                    ptt = psS.tile([128, 512], f32, tag="s",
                                   name=f"ptt{arg}")
                    theta_half(arg, ptt, True)
                else:
                    pgt = psS.tile([128, 512], f32, tag="s",
                                   name=f"pgt{arg}")
                    gt_group(arg, pgt, True)

            trail = {0: ("th", 2), 2: ("th", 3), 4: ("g", 2), 6: ("g", 3),
                     8: ("th", 4), 10: ("th", 5), 12: ("g", 4),
                     14: ("g", 5), 16: ("th", 6), 18: ("th", 7),
                     20: ("g", 6), 22: ("g", 7)}

            for nt in range(NT):
                th_nt = theta[:, nt * 128:(nt + 1) * 128]

                def s_chunk(t):
                    spt = psS.tile([128, 512], f32, tag="s")
                    cols = slice(t * 512, (t + 1) * 512)
                    _tag(nc.tensor.matmul(spt[:, :], th_nt, phi[:, cols],
                                          start=True, stop=True),
                         f"Smm.{nt}.{t}")
                    return spt

                # z-chain of nt-2, entirely on DVE with >=1-iteration-old
                # inputs: never stalls regardless of scheduler placement
                if nt >= 2:
                    j2 = nt - 2
                    emit_chain(j2)
                sp = [s_chunk(t) for t in range(4)]
                if nt >= 2:
                    emit_Y(nt - 2)
                bb = bbp.tile([128, 2048], i16, tag="bb")
                bv = bb.bitcast(bf16)
                for t in range(2):
                    _tag(nc.vector.tensor_scalar(
                        bb[:, t * 512:(t + 1) * 512], sp[t][:, :],
                        SCH_A, SCH_B, ALU.mult, ALU.add), f"sch.{nt}.{t}")
                # Pool folds: two independent 512 pairs — both complete
                # ~1.3us after sch3, so the lag-2 DVE ttr never waits
                scr = scp.tile([128, 1024], bf16, tag="scr")
                _tag(nc.gpsimd.tensor_tensor(
                    scr[:, 0:512], bv[:, 0:512], bv[:, 512:1024],
                    ALU.add), f"fold.{nt}")
                for t in range(2, 4):
                    _tag(nc.vector.tensor_scalar(
                        bb[:, t * 512:(t + 1) * 512], sp[t][:, :],
                        SCH_A, SCH_B, ALU.mult, ALU.add), f"sch.{nt}.{t}")
                _tag(nc.gpsimd.tensor_tensor(
                    scr[:, 512:1024], bv[:, 1024:1536], bv[:, 1536:2048],
                    ALU.add), f"foldB.{nt}")
                if nt in trail:
                    trailing(trail[nt])
                zc = zcp.tile([128, 6], f32, tag="zc")
                ft, off = fdst(nt)
                for t in range(4):
                    spe = s_chunk(4 + t)
                    _tag(nc.scalar.activation(
                        ft[:, off + t * 512:off + (t + 1) * 512],
                        spe[:, :],
                        AF.Exp, bias=actb[:, :], scale=1.0,
                        accum_out=zc[:, 1 + t:2 + t]), f"exp.{nt}.{t}")
                fold_st[nt] = (scr, zc, bv)
                st[nt] = bb
            # drain the pipelined chains for the last two nt
            emit_chain(NT - 2)
            emit_chain(NT - 1)

        # x/theta/phi no longer needed
        es01.close()

        # ===== phase 2: windows + y drains + wy stats =====
        with tc.tile_pool(name="ysbp", bufs=1) as ysbp:
            ysb = ysbp.tile([CI, N], bf16)
            wys = [ysbp.tile([128, N], bf16, name=f"wys{i}")
                   for i in range(2)]
            sqd = ysbp.tile([128, 1], f32)

            es_y1 = ExitStack()
            psY1 = es_y1.enter_context(
                tc.tile_pool(name="psY1", bufs=1, space="PSUM"))
            y1 = psY1.tile([CI, M1], f32)

            def y1_window(w):
                ws = slice(w * 512, (w + 1) * 512)
                for nt in range(NT):
                    ft, off = fdst(nt)
                    g_nt = gts[:, nt * CI:(nt + 1) * CI]
                    nc.tensor.matmul(
                        y1[:, ws], g_nt,
                        ft[:, off + w * 512:off + (w + 1) * 512],
                        start=(nt == 0), stop=(nt == NT - 1))

            # w0 first: its tail covers the wait for gscale(30/31)
            # before Y(30)/Y(31)
            y1_window(0)
            emit_Y(NT - 2)
            emit_Y(NT - 1)
            es_bb.close()
            nc.scalar.activation(ysb[:, 0:1024], y0[:, 0:1024],
                                 AF.Identity, scale=1.0)
            # switch ACT to the sqrt/identity table now, off the BN
            # critical path (identity stays resident in both sets)
            nc.scalar.activation(sqd[:, :], epsb[:, :], AF.Sqrt, scale=1.0)
            nc.vector.tensor_copy(ysb[:, 1024:2048], y0[:, 1024:2048])
            es_y0.close()

            with tc.tile_pool(name="psW", bufs=2, space="PSUM") as psW, \
                 tc.tile_pool(name="ttp", bufs=2) as ttp:

                def wy_chunk(cth, pc):
                    # wy (stored for phase 3) + stats for cols
                    # pc*1024:(pc+1)*1024, ct-half cth
                    wsl = slice(cth * 128, (cth + 1) * 128)
                    wp = psW.tile([128, 1024], f32, tag="w")
                    for h in range(2):
                        ys = slice(pc * 1024 + h * 512,
                                   pc * 1024 + (h + 1) * 512)
                        nc.tensor.matmul(
                            wp[:, h * 512:(h + 1) * 512],
                            wwb[:, wsl], ysb[:, ys],
                            start=True, stop=True)
                    col = cth * 4 + pc
                    wy = wys[cth][:, pc * 1024:(pc + 1) * 1024]
                    nc.scalar.activation(
                        wy, wp[:, :], AF.Identity,
                        bias=pf[:, 2 + cth:3 + cth], scale=1.0,
                        accum_out=s1c[:, col:col + 1])
                    jk2 = ttp.tile([128, 1024], bf16, tag="j2")
                    nc.vector.tensor_tensor(jk2[:, :], wy, wy, ALU.mult)
                    nc.vector.reduce_sum(s2c[:, col:col + 1], jk2[:, :],
                                         axis=AX.X)

                wy_chunk(0, 0)
                wy_chunk(1, 0)
                y1_window(1)
                nc.scalar.activation(ysb[:, M0:M0 + 512], y1[:, 0:512],
                                     AF.Identity, scale=1.0)
                wy_chunk(0, 1)
                y1_window(2)
                nc.vector.tensor_copy(ysb[:, M0 + 512:M0 + 1024],
                                      y1[:, 512:1024])
                wy_chunk(1, 1)
                wy_chunk(0, 2)
                y1_window(3)
                nc.scalar.activation(ysb[:, M0 + 1024:M0 + 1536],
                                     y1[:, 1024:1536],
                                     AF.Identity, scale=1.0)
                wy_chunk(1, 2)
                nc.vector.tensor_copy(ysb[:, M0 + 1536:N],
                                      y1[:, 1536:2048])
                wy_chunk(0, 3)
                wy_chunk(1, 3)
                for i, (src2, col) in enumerate(
                        [(s1c, 0), (s1c, 4), (s2c, 0), (s2c, 4)]):
                    nc.vector.reduce_sum(stats[:, i:i + 1],
                                         src2[:, col:col + 4], axis=AX.X)
            es_y1.close()

            # ====== phase 3: all-reduce + finalize ======
            with tc.tile_pool(name="fin", bufs=1) as fp2, \
                 tc.tile_pool(name="obuf", bufs=2) as obp, \
                 tc.tile_pool(name="obuf2", bufs=4) as obp2:
                nc.sync.dma_start(cc_in[:, :], stats[:, :])
                if no_collective:
                    nc.sync.dma_start(cc_out[:, :], cc_in[:, :])
                else:
                    nc.gpsimd.collective_compute(
                        "AllReduce", mybir.AluOpType.add,
                        replica_groups=[list(range(n_cores))],
                        ins=[cc_in.opt()], outs=[cc_out.opt()])
                nc.sync.dma_start(statsg[:, :], cc_out[:, :])

                inv = 1.0 / (B * N)
                mean2 = fp2.tile([128, 2], f32)
                e2 = fp2.tile([128, 2], f32)
                var2 = fp2.tile([128, 2], f32)
                sq = fp2.tile([128, 2], f32)
                rstd = fp2.tile([128, 2], f32)
                acol = fp2.tile([128, 2], f32)
                btot = fp2.tile([128, 2], f32)
                nc.vector.tensor_scalar_mul(mean2[:, :], statsg[:, 0:2], inv)
                nc.vector.tensor_scalar_mul(e2[:, :], statsg[:, 2:4], inv)
                nc.vector.tensor_mul(var2[:, :], mean2[:, :], mean2[:, :])
                nc.vector.tensor_sub(var2[:, :], e2[:, :], var2[:, :])
                nc.scalar.activation(sq[:, :], var2[:, :], AF.Sqrt,
                                     bias=epsb[:, :], scale=1.0)
                nc.vector.reciprocal(rstd[:, :], sq[:, :])
                nc.vector.tensor_mul(acol[:, :], rstd[:, :], pf[:, 4:6])
                # wy was stored with the conv bias: out = acol*wy + btot + l
                # with btot = beta - mean*acol
                nc.vector.tensor_mul(btot[:, :], mean2[:, :], acol[:, :])
                nc.vector.tensor_sub(btot[:, :], pf[:, 6:8], btot[:, :])
                # scale+bias: chunks 0/2 on ACT, 1/3 on DVE (bf16 2x);
                # residual adds: DVE, except chunk 1 on Pool
                for i, (cth, pc) in enumerate(
                        [(0, 0), (0, 1), (1, 0), (1, 1)]):
                    wsl = slice(cth * 128, (cth + 1) * 128)
                    psl = slice(pc * 2048, (pc + 1) * 2048)
                    ob = obp.tile([128, 2048], bf16, tag="ob")
                    if i % 2 == 0:
                        nc.scalar.activation(
                            ob[:, :], wys[cth][:, psl], AF.Identity,
                            bias=btot[:, cth:cth + 1],
                            scale=acol[:, cth:cth + 1])
                    else:
                        nc.vector.tensor_scalar(
                            ob[:, :], wys[cth][:, psl],
                            acol[:, cth:cth + 1], btot[:, cth:cth + 1],
                            ALU.mult, ALU.add)
                    ob2 = obp2.tile([128, 2048], bf16, tag="o2")
                    eng = nc.gpsimd if i == 1 else nc.vector
                    eng.tensor_tensor(ob2[:, :], ob[:, :],
                                      lt[cth][pc][:, :], ALU.add)
                    (nc.sync if i % 2 == 0 else nc.scalar).dma_start(
                        out[wsl, psl], ob2[:, :])

    nc.compile()
    return nc


def _get_nc(n_cores: int):
    if n_cores not in _CACHE:
        _CACHE[n_cores] = _build(n_cores)
    return _CACHE[n_cores]


def make_in_maps(inputs: dict, n_cores: int = N_CORES):
    """Build per-core input maps from full-size inputs."""
    f = np.float32
    bf = ml_dtypes.bfloat16
    x = np.ascontiguousarray(inputs["x"], f).reshape(B, CS, N)
    l = np.ascontiguousarray(inputs["l"], f).reshape(B, CT, N)
    thwT = np.ascontiguousarray(inputs["theta_w"].T, f)   # [CS, CI]
    phwT = np.ascontiguousarray(inputs["phi_w"].T, f)
    gwT = np.asarray(inputs["g_w"].T, f)                  # [CS, CI]
    wwT = np.asarray(inputs["w_w"].T, f)                  # [CI, CT]
    pwb = np.concatenate([thwT[0:128], thwT[128:256],
                          phwT[0:128], phwT[128:256],
                          gwT[0:128], gwT[128:256], wwT], axis=1)
    pf = np.zeros((128, 10), f)
    pf[:, 0] = np.asarray(inputs["theta_b"], f)
    pf[:, 1] = np.asarray(inputs["phi_b"], f)
    for cth in range(2):
        hsl = slice(cth * 128, (cth + 1) * 128)
        pf[:, 2 + cth] = np.asarray(inputs["w_b"], f)[hsl]
        pf[:, 4 + cth] = np.asarray(inputs["bn_gamma"], f)[hsl]
        pf[:, 6 + cth] = np.asarray(inputs["bn_beta"], f)[hsl]
    shared = {
        "pwb": np.ascontiguousarray(pwb).astype(bf),
        "pf": pf,
        "g_b_bf": np.asarray(inputs["g_b"], f).reshape(1, CI).astype(bf),
    }
    return [{"x": x[i].astype(bf), "lres": l[i].astype(bf), **shared}
            for i in range(n_cores)]


def kernel(**inputs) -> np.ndarray:
    from concourse import bass_utils

    nc = _get_nc(N_CORES)
    in_maps = make_in_maps(inputs, N_CORES)
    res = bass_utils.run_bass_kernel_spmd(
        nc, in_maps, core_ids=list(range(N_CORES)))
    outs = [np.asarray(res.results[i]["out"], dtype=np.float32)
            for i in range(N_CORES)]
    return np.stack(outs, 0).reshape(B, CT, 64, 64)


if __name__ == "__main__":
    nc = _get_nc(1)
    print("build+compile OK")


# revision 41
# speedup vs baseline: 1.0530x; 1.0530x over previous
"""Trainium2 Bass kernel for CrossNonLocalBlock — v4 (engine-rebalanced).

Shapes (hardcoded): B=8, Cs=Ct=256, Ci=128, H=W=64 (N=4096 spatial).
Sharding: data-parallel over batch (1 batch element per NeuronCore, 8 cores);
params replicated; BN batch statistics all-reduced in-kernel.

Per-core structure:
  phase 0: one packed param DMA + 8 big x/l DMAs (l stays resident for the
           phase-3 residual); phi fully + theta c0 + g groups 0/1 up front,
           remaining theta chunks / g groups trail into early nt iterations
           through the S-ring PSUM slots.
  phase 1: attention n-loop.  Per nt: 8 S matmuls (512-col) into a 2-slot
           PSUM ring; DVE "schraudolph-16" exp on cols 0:2048 (2 ops,
           int16 bits == bf16(exp)); ACT exp on cols 2048:4096 (2 ops,
           fused row-sum accum); Pool folds the sch halves, DVE
           tensor_tensor_reduce finishes the row-sum, Pool combines z and
           scales g by 1/z; Y matmuls for cols 0:2048 accumulate in PSUM
           at lag 2.
  phase 2: window-major Y matmuls for cols 2048:4096 from stored f, with
           y0/y1 drains and W-conv statistics chunks (ACT accum for S1,
           fused DVE square+reduce for S2) pipelined inside.
  phase 3: 2KB AllReduce of BN stats with W-conv recompute prefired under
           it; BN normalize + residual (resident l tiles) + bf16 stores.

Pool lifetimes use per-side allocation stacks (SBUF/PSUM each have left and
right stacks) plus ExitStacks so psY0 (right) can close mid-phase-2 while
psY1 (left) stays live, and the sch-bits ring (SBUF right) survives into
phase 2 for the last two Y matmul groups.
"""

import sys
from contextlib import ExitStack

import numpy as np
import ml_dtypes

if "/opt/trn_rl_repo" not in sys.path:
    sys.path.insert(0, "/opt/trn_rl_repo")

B, CS, CT, CI, N = 8, 256, 256, 128, 4096
NT = N // 128          # 32 n-tiles
M0 = 2048              # PSUM-resident Y columns (sch16 cols)
M1 = N - M0            # stored-f columns (ACT exp cols)
BN_EPS = 1e-5
N_CORES = 8

# schraudolph-16: bf16bits(exp(x)) ~= int16(x * 128/ln2 + (127*128 - 5.5)).
# B is recentered (+249.5) so the measured logit range [-84.4, 80.6] sits
# mid-window (int16 bits must stay in (0, 32767): S in (-89.3, 88.1)).  That
# scales f by 2^(249.5/128); the ACT-exp chunks get the matching ln-space
# bias so row-sums stay consistent (softmax cancels the common factor).
SCH_A = 128.0 / float(np.log(2.0))
SCH_B = 16500.0
ACT_BIAS = (SCH_B - (127.0 * 128.0 - 5.5)) * float(np.log(2.0)) / 128.0

_CACHE = {}
ROLES = {}


def _tag(inst, role):
    try:
        ROLES[inst.ins.name] = role
    except Exception:
        pass
    return inst


def _build(n_cores: int, no_collective: bool = False):
    import concourse.mybir as mybir
    import concourse.tile as tile
    from concourse import bacc

    f32 = mybir.dt.float32
    bf16 = mybir.dt.bfloat16
    i16 = mybir.dt.int16
    AF = mybir.ActivationFunctionType
    AX = mybir.AxisListType
    ALU = mybir.AluOpType

    nc = bacc.Bacc("TRN2", target_bir_lowering=False, debug=False,
                   num_devices=n_cores)

    # ---- DRAM I/O (per-core) ----
    # pwb packed bf16 [128, 1024]: thw k0|k1 | phw k0|k1 | gw k0|k1 | wwT
    pwb_d = nc.dram_tensor("pwb", [128, 1024], bf16, kind="ExternalInput").ap()
    # pf: packed f32 [128, 10]: thb|phb|wb0|wb1|gam0|gam1|bet0|bet1|-|-
    pf_d = nc.dram_tensor("pf", [128, 10], f32, kind="ExternalInput").ap()
    gbr_d = nc.dram_tensor("g_b_bf", [1, CI], bf16, kind="ExternalInput").ap()
    x_d = nc.dram_tensor("x", [CS, N], bf16, kind="ExternalInput").ap()
    l_d = nc.dram_tensor("lres", [CT, N], bf16, kind="ExternalInput").ap()
    out = nc.dram_tensor("out", [CT, N], bf16, kind="ExternalOutput").ap()

    with tile.TileContext(nc) as tc, \
         tc.tile_pool(name="persist", bufs=1) as pp, \
         tc.tile_pool(name="dram", bufs=1, space="DRAM") as dp, \
         tc.tile_pool(name="fstore", bufs=1) as fsp:
        gts = pp.tile([128, NT * CI], bf16)  # gT tiles (scaled in-loop) 8KB
        pwb = pp.tile([128, 1024], bf16)     # 2KB
        pf = pp.tile([128, 10], f32)
        gbr = pp.tile([1, CI], bf16)
        onesb = pp.tile([1, 128], bf16)
        stats = pp.tile([128, 4], f32)
        statsg = pp.tile([128, 4], f32)
        s1c = pp.tile([128, 8], f32)
        s2c = pp.tile([128, 8], f32)
        actb = pp.tile([128, 1], f32)
        epsb = pp.tile([128, 1], f32)
        # l resident: [k half][col half 2048] — reused for the residual
        lt = [[pp.tile([128, 2048], bf16, name=f"lt{k}{h}")
               for h in range(2)] for k in range(2)]

        thw = pwb[:, 0:2 * CI]
        phw = pwb[:, 2 * CI:4 * CI]
        gwb = pwb[:, 4 * CI:6 * CI]
        wwb = pwb[:, 6 * CI:8 * CI]
        thb = pf[:, 0:1]
        phb = pf[:, 1:2]

        cc_in = dp.tile([128, 4], f32)
        cc_out = dp.tile([128, 4], f32,
                         addr_space="Shared" if n_cores > 1 else "Local")

        fst = [fsp.tile([128, 8 * M1], bf16, name=f"fst{i}")
               for i in range(4)]

        def fdst(nt):
            return fst[nt // 8], (nt % 8) * M1

        # x/theta/phi live from phase 0 through the end of the n-loop
        es01 = ExitStack()
        p01 = es01.enter_context(tc.tile_pool(name="p01", bufs=1))
        theta = p01.tile([CI, N], bf16)      # 8KB/part
        phi = p01.tile([CI, N], bf16)        # 8KB/part
        xt = [[p01.tile([128, 2048], bf16, name=f"x{k}{h}")
               for h in range(2)] for k in range(2)]

        nc.vector.memset(onesb[:, :], 1.0)
        nc.vector.memset(actb[:, :], ACT_BIAS)
        nc.vector.memset(epsb[:, :], BN_EPS)

        # ---- DMAs: params first (biases gate every projection drain),
        #      x/l spread across 4 engine queues so HWDGE gen and the
        #      transfers run in parallel
        nc.sync.dma_start(pwb[:, :], pwb_d[:, :])
        nc.scalar.dma_start(pf[:, :], pf_d[:, :])
        nc.scalar.dma_start(gbr[:, :], gbr_d[:, :])
        # l h0 first: phi gates the attention loop
        nc.sync.dma_start(lt[0][0][:, :], l_d[0:128, 0:2048])
        nc.scalar.dma_start(lt[1][0][:, :], l_d[128:256, 0:2048])
        nc.sync.dma_start(xt[0][0][:, :], x_d[0:128, 0:2048])
        nc.scalar.dma_start(xt[1][0][:, :], x_d[128:256, 0:2048])
        nc.sync.dma_start(lt[0][1][:, :], l_d[0:128, 2048:N])
        nc.scalar.dma_start(lt[1][1][:, :], l_d[128:256, 2048:N])
        # x h1 feeds only the trailing projection units (nt >= 8)
        nc.sync.dma_start(xt[0][1][:, :], x_d[0:128, 2048:N])
        nc.scalar.dma_start(xt[1][1][:, :], x_d[128:256, 2048:N])

        # preload the exp/identity activation table during the DMA wait,
        # and keep the PE busy with dummy matmuls so it reaches full clock
        # before the first projection matmul
        with tc.tile_pool(name="dum", bufs=1) as dum, \
             tc.tile_pool(name="dumps", bufs=1, space="PSUM") as dumps:
            tdum = dum.tile([128, 1], f32)
            nc.scalar.activation(tdum[:, :], actb[:, :], AF.Exp, scale=1.0)
            jmp = dumps.tile([128, 128], f32)
            for _ in range(48):
                nc.tensor.matmul(jmp[:, :], onesb[:, :], onesb[:, :],
                                 start=True, stop=True)

        # ======== projection helpers (PSUM tile passed in) ========
        def theta_half(c2, pt, on_act):
            # 512-col theta slab c2 in 0..7
            csl = slice(c2 * 512, (c2 + 1) * 512)
            xs = slice((c2 % 4) * 512, (c2 % 4 + 1) * 512)
            for k in range(2):
                nc.tensor.matmul(
                    pt[:, :], thw[:, k * CI:(k + 1) * CI],
                    xt[k][c2 // 4][:, xs],
                    start=(k == 0), stop=(k == 1))
            if on_act:
                nc.scalar.activation(theta[:, csl], pt[:, :],
                                     AF.Identity, bias=thb, scale=1.0)
            else:
                nc.vector.tensor_scalar(theta[:, csl], pt[:, :],
                                        thb, None, ALU.add)

        def phi_chunk(c, pl, on_act):
            csl = slice(c * 1024, (c + 1) * 1024)
            loc = (c % 2) * 1024
            for h in range(2):
                hs = slice(h * 512, (h + 1) * 512)
                ls = slice(loc + h * 512, loc + (h + 1) * 512)
                for k in range(2):
                    nc.tensor.matmul(
                        pl[:, hs], phw[:, k * CI:(k + 1) * CI],
                        lt[k][c // 2][:, ls],
                        start=(k == 0), stop=(k == 1))
            if on_act:
                nc.scalar.activation(phi[:, csl], pl[:, :],
                                     AF.Identity, bias=phb, scale=1.0)
            else:
                nc.vector.tensor_scalar(phi[:, csl], pl[:, :],
                                        phb, None, ALU.add)

        def gt_group(grp, pg, on_act):
            # 4 nt tiles -> one [128,512] PSUM -> gts (bf16)
            h = grp // 4
            for j in range(4):
                nt = grp * 4 + j
                lo = (nt % 16) * 128
                jsl = slice(j * 128, (j + 1) * 128)
                nc.tensor.matmul(pg[:, jsl], xt[0][h][:, lo:lo + 128],
                                 gwb[:, 0:CI], start=True, stop=False)
                nc.tensor.matmul(pg[:, jsl], xt[1][h][:, lo:lo + 128],
                                 gwb[:, CI:2 * CI], start=False, stop=False)
                nc.tensor.matmul(pg[:, jsl], onesb[:, :], gbr[:, :],
                                 start=False, stop=True)
            gsl = slice(grp * 512, (grp + 1) * 512)
            if on_act:
                nc.scalar.activation(gts[:, gsl], pg[:, :],
                                     AF.Identity, scale=1.0)
            else:
                nc.vector.tensor_copy(gts[:, gsl], pg[:, :])

        # ============ phase 0 (up-front part) ============
        with tc.tile_pool(name="ps0", bufs=2, space="PSUM") as ps0, \
             tc.tile_pool(name="psg", bufs=2, space="PSUM") as psg:
            pp0 = ps0.tile([128, 1024], f32, tag="p0")
            phi_chunk(0, pp0, False)
            pp1 = ps0.tile([128, 1024], f32, tag="p0")
            phi_chunk(1, pp1, True)
            pt0 = psg.tile([128, 512], f32, tag="pg")
            theta_half(0, pt0, True)
            pt1 = psg.tile([128, 512], f32, tag="pg")
            theta_half(1, pt1, False)
            pg0 = psg.tile([128, 512], f32, tag="pg")
            gt_group(0, pg0, False)
            pg1 = psg.tile([128, 512], f32, tag="pg")
            gt_group(1, pg1, True)
            pp2 = ps0.tile([128, 1024], f32, tag="p0")
            phi_chunk(2, pp2, True)
            pp3 = ps0.tile([128, 1024], f32, tag="p0")
            phi_chunk(3, pp3, False)

        # ============ phase 1: attention n-loop ============
        es_y0 = ExitStack()
        psY0 = es_y0.enter_context(
            tc.tile_pool(name="psY0", bufs=1, space="PSUM", side="right"))
        y0 = psY0.tile([CI, M0], f32)

        # sch-bits ring + small per-nt tiles: SBUF right stack so they can
        # outlive p01 (closed right after the loop) into early phase 2
        es_bb = ExitStack()
        bbp = es_bb.enter_context(
            tc.tile_pool(name="bbp", bufs=3, side="right"))
        scp = es_bb.enter_context(
            tc.tile_pool(name="scp", bufs=3, side="right"))
        jkp = es_bb.enter_context(
            tc.tile_pool(name="jkp", bufs=1, side="right"))
        zcp = es_bb.enter_context(
            tc.tile_pool(name="zcp", bufs=3, side="right"))
        zp = es_bb.enter_context(
            tc.tile_pool(name="zp", bufs=3, side="right"))

        st = {}
        chain_st = {}
        fold_st = {}

        def emit_chain(j):
            # full z-chain for nt j on DVE: fold(j) completed >=1 full
            # iteration ago, ACT accums too
            scr_p, zc_p, bv_p = fold_st.pop(j)
            jk = jkp.tile([128, 512], bf16, tag="jk")
            _tag(nc.vector.tensor_tensor(
                jk[:, :], scr_p[:, 0:512], scr_p[:, 512:1024], ALU.add),
                 f"ttf.{j}")
            _tag(nc.vector.reduce_sum(zc_p[:, 0:1], jk[:, :], axis=AX.X),
                 f"ttr.{j}")
            z2 = zp.tile([128, 1], f32, tag="z2")
            _tag(nc.vector.reduce_sum(z2[:, :], zc_p[:, 0:5], axis=AX.X),
                 f"z.{j}")
            rz = zp.tile([128, 1], f32, tag="rz")
            nc.vector.reciprocal(rz[:, :], z2[:, :])
            g_p = gts[:, j * CI:(j + 1) * CI]
            _tag(nc.vector.tensor_scalar(
                g_p, g_p, rz[:, :], None, ALU.mult), f"gdiv.{j}")

        def emit_Y(j):
            bb = st.pop(j)
            bv = bb.bitcast(bf16)
            g_j = gts[:, j * CI:(j + 1) * CI]
            for w in range(4):
                _tag(nc.tensor.matmul(
                    y0[:, w * 512:(w + 1) * 512],
                    g_j, bv[:, w * 512:(w + 1) * 512],
                    start=(j == 0), stop=(j == NT - 1)), f"Ymm.{j}.{w}")

        with tc.tile_pool(name="psS", bufs=4, space="PSUM") as psS:
            # trailing phase-0 units: emitted inside early nt iterations
            # through the S-ring slots
            def trailing(slot):
                kind, arg = slot
                if kind == "th":
                    ptt = psS.tile([128, 512], f32, tag="s",
                                   name=f"ptt{arg}")
                    theta_half(arg, ptt, True)
                else:
                    pgt = psS.tile([128, 512], f32, tag="s",
                                   name=f"pgt{arg}")
                    gt_group(arg, pgt, True)

            trail = {0: ("th", 2), 2: ("th", 3), 4: ("g", 2), 6: ("g", 3),
                     8: ("th", 4), 10: ("th", 5), 12: ("g", 4),
                     14: ("g", 5), 16: ("th", 6), 18: ("th", 7),
                     20: ("g", 6), 22: ("g", 7)}

            for nt in range(NT):
                th_nt = theta[:, nt * 128:(nt + 1) * 128]

                def s_chunk(t):
                    spt = psS.tile([128, 512], f32, tag="s")
                    cols = slice(t * 512, (t + 1) * 512)
                    _tag(nc.tensor.matmul(spt[:, :], th_nt, phi[:, cols],
                                          start=True, stop=True),
                         f"Smm.{nt}.{t}")
                    return spt

                # z-chain of nt-2, entirely on DVE with >=1-iteration-old
                # inputs: never stalls regardless of scheduler placement
                if nt >= 2:
                    j2 = nt - 2
                    emit_chain(j2)
                sp = [s_chunk(t) for t in range(4)]
                if nt >= 2:
                    emit_Y(nt - 2)
                bb = bbp.tile([128, 2048], i16, tag="bb")
                bv = bb.bitcast(bf16)
                for t in range(2):
                    _tag(nc.vector.tensor_scalar(
                        bb[:, t * 512:(t + 1) * 512], sp[t][:, :],
                        SCH_A, SCH_B, ALU.mult, ALU.add), f"sch.{nt}.{t}")
                # Pool folds: two independent 512 pairs — both complete
                # ~1.3us after sch3, so the lag-2 DVE ttr never waits
                scr = scp.tile([128, 1024], bf16, tag="scr")
                _tag(nc.gpsimd.tensor_tensor(
                    scr[:, 0:512], bv[:, 0:512], bv[:, 512:1024],
                    ALU.add), f"fold.{nt}")
                for t in range(2, 4):
                    _tag(nc.vector.tensor_scalar(
                        bb[:, t * 512:(t + 1) * 512], sp[t][:, :],
                        SCH_A, SCH_B, ALU.mult, ALU.add), f"sch.{nt}.{t}")
                _tag(nc.gpsimd.tensor_tensor(
                    scr[:, 512:1024], bv[:, 1024:1536], bv[:, 1536:2048],
                    ALU.add), f"foldB.{nt}")
                if nt in trail:
                    trailing(trail[nt])
                zc = zcp.tile([128, 6], f32, tag="zc")
                ft, off = fdst(nt)
                for t in range(4):
                    spe = s_chunk(4 + t)
                    _tag(nc.scalar.activation(
                        ft[:, off + t * 512:off + (t + 1) * 512],
                        spe[:, :],
                        AF.Exp, bias=actb[:, :], scale=1.0,
                        accum_out=zc[:, 1 + t:2 + t]), f"exp.{nt}.{t}")
                fold_st[nt] = (scr, zc, bv)
                st[nt] = bb
            # drain the pipelined chains for the last two nt
            emit_chain(NT - 2)
            emit_chain(NT - 1)

        # x/theta/phi no longer needed
        es01.close()

        # ===== phase 2: windows + y drains + wy stats =====
        with tc.tile_pool(name="ysbp", bufs=1) as ysbp:
            ysb = ysbp.tile([CI, N], bf16)
            wys = [ysbp.tile([128, N], bf16, name=f"wys{i}")
                   for i in range(2)]
            sqd = ysbp.tile([128, 1], f32)

            es_y1 = ExitStack()
            psY1 = es_y1.enter_context(
                tc.tile_pool(name="psY1", bufs=1, space="PSUM"))
            y1 = psY1.tile([CI, M1], f32)

            def y1_window(w):
                ws = slice(w * 512, (w + 1) * 512)
                for nt in range(NT):
                    ft, off = fdst(nt)
                    g_nt = gts[:, nt * CI:(nt + 1) * CI]
                    nc.tensor.matmul(
                        y1[:, ws], g_nt,
                        ft[:, off + w * 512:off + (w + 1) * 512],
                        start=(nt == 0), stop=(nt == NT - 1))

            # w0 first: its tail covers the wait for gscale(30/31)
            # before Y(30)/Y(31)
            y1_window(0)
            emit_Y(NT - 2)
            emit_Y(NT - 1)
            es_bb.close()
            nc.scalar.activation(ysb[:, 0:1024], y0[:, 0:1024],
                                 AF.Identity, scale=1.0)
            # switch ACT to the sqrt/identity table now, off the BN
            # critical path (identity stays resident in both sets)
            nc.scalar.activation(sqd[:, :], epsb[:, :], AF.Sqrt, scale=1.0)
            nc.vector.tensor_copy(ysb[:, 1024:2048], y0[:, 1024:2048])
            es_y0.close()

            with tc.tile_pool(name="psW", bufs=2, space="PSUM") as psW, \
                 tc.tile_pool(name="ttp", bufs=2) as ttp:

                def wy_chunk(cth, pc):
                    # wy (stored for phase 3) + stats for cols
                    # pc*1024:(pc+1)*1024, ct-half cth
                    wsl = slice(cth * 128, (cth + 1) * 128)
                    wp = psW.tile([128, 1024], f32, tag="w")
                    for h in range(2):
                        ys = slice(pc * 1024 + h * 512,
                                   pc * 1024 + (h + 1) * 512)
                        nc.tensor.matmul(
                            wp[:, h * 512:(h + 1) * 512],
                            wwb[:, wsl], ysb[:, ys],
                            start=True, stop=True)
                    col = cth * 4 + pc
                    wy = wys[cth][:, pc * 1024:(pc + 1) * 1024]
                    nc.scalar.activation(
                        wy, wp[:, :], AF.Identity,
                        bias=pf[:, 2 + cth:3 + cth], scale=1.0,
                        accum_out=s1c[:, col:col + 1])
                    jk2 = ttp.tile([128, 1024], bf16, tag="j2")
                    nc.vector.tensor_tensor(jk2[:, :], wy, wy, ALU.mult)
                    nc.vector.reduce_sum(s2c[:, col:col + 1], jk2[:, :],
                                         axis=AX.X)

                wy_chunk(0, 0)
                wy_chunk(1, 0)
                y1_window(1)
                nc.scalar.activation(ysb[:, M0:M0 + 512], y1[:, 0:512],
                                     AF.Identity, scale=1.0)
                wy_chunk(0, 1)
                y1_window(2)
                nc.vector.tensor_copy(ysb[:, M0 + 512:M0 + 1024],
                                      y1[:, 512:1024])
                wy_chunk(1, 1)
                wy_chunk(0, 2)
                y1_window(3)
                nc.scalar.activation(ysb[:, M0 + 1024:M0 + 1536],
                                     y1[:, 1024:1536],
                                     AF.Identity, scale=1.0)
                wy_chunk(1, 2)
                nc.vector.tensor_copy(ysb[:, M0 + 1536:N],
                                      y1[:, 1536:2048])
                wy_chunk(0, 3)
                wy_chunk(1, 3)
                for i, (src2, col) in enumerate(
                        [(s1c, 0), (s1c, 4), (s2c, 0), (s2c, 4)]):
                    nc.vector.reduce_sum(stats[:, i:i + 1],
                                         src2[:, col:col + 4], axis=AX.X)
            es_y1.close()

            # ====== phase 3: all-reduce + finalize ======
            with tc.tile_pool(name="fin", bufs=1) as fp2, \
                 tc.tile_pool(name="obuf", bufs=2) as obp, \
                 tc.tile_pool(name="obuf2", bufs=4) as obp2:
                nc.sync.dma_start(cc_in[:, :], stats[:, :])
                if no_collective:
                    nc.sync.dma_start(cc_out[:, :], cc_in[:, :])
                else:
                    nc.gpsimd.collective_compute(
                        "AllReduce", mybir.AluOpType.add,
                        replica_groups=[list(range(n_cores))],
                        ins=[cc_in.opt()], outs=[cc_out.opt()])
                nc.sync.dma_start(statsg[:, :], cc_out[:, :])

                inv = 1.0 / (B * N)
                mean2 = fp2.tile([128, 2], f32)
                e2 = fp2.tile([128, 2], f32)
                var2 = fp2.tile([128, 2], f32)
                sq = fp2.tile([128, 2], f32)
                rstd = fp2.tile([128, 2], f32)
                acol = fp2.tile([128, 2], f32)
                btot = fp2.tile([128, 2], f32)
                nc.vector.tensor_scalar_mul(mean2[:, :], statsg[:, 0:2], inv)
                nc.vector.tensor_scalar_mul(e2[:, :], statsg[:, 2:4], inv)
                nc.vector.tensor_mul(var2[:, :], mean2[:, :], mean2[:, :])
                nc.vector.tensor_sub(var2[:, :], e2[:, :], var2[:, :])
                nc.scalar.activation(sq[:, :], var2[:, :], AF.Sqrt,
                                     bias=epsb[:, :], scale=1.0)
                nc.vector.reciprocal(rstd[:, :], sq[:, :])
                nc.vector.tensor_mul(acol[:, :], rstd[:, :], pf[:, 4:6])
                # wy was stored with the conv bias: out = acol*wy + btot + l
                # with btot = beta - mean*acol
                nc.vector.tensor_mul(btot[:, :], mean2[:, :], acol[:, :])
                nc.vector.tensor_sub(btot[:, :], pf[:, 6:8], btot[:, :])
                # scale+bias: chunks 0/2 on ACT, 1/3 on DVE (bf16 2x);
                # residual adds: DVE, except chunk 1 on Pool
                for i, (cth, pc) in enumerate(
                        [(0, 0), (0, 1), (1, 0), (1, 1)]):
                    wsl = slice(cth * 128, (cth + 1) * 128)
                    psl = slice(pc * 2048, (pc + 1) * 2048)
                    ob = obp.tile([128, 2048], bf16, tag="ob")
                    if i % 2 == 0:
                        nc.scalar.activation(
                            ob[:, :], wys[cth][:, psl], AF.Identity,
                            bias=btot[:, cth:cth + 1],
                            scale=acol[:, cth:cth + 1])
                    else:
                        nc.vector.tensor_scalar(
                            ob[:, :], wys[cth][:, psl],
                            acol[:, cth:cth + 1], btot[:, cth:cth + 1],
                            ALU.mult, ALU.add)
                    ob2 = obp2.tile([128, 2048], bf16, tag="o2")
                    eng = nc.gpsimd if i == 1 else nc.vector
                    eng.tensor_tensor(ob2[:, :], ob[:, :],
                                      lt[cth][pc][:, :], ALU.add)
                    (nc.sync if i % 2 == 0 else nc.scalar).dma_start(
                        out[wsl, psl], ob2[:, :])

    nc.compile()
    return nc


def _get_nc(n_cores: int):
    if n_cores not in _CACHE:
        _CACHE[n_cores] = _build(n_cores)
    return _CACHE[n_cores]


def make_in_maps(inputs: dict, n_cores: int = N_CORES):
    """Build per-core input maps from full-size inputs."""
    f = np.float32
    bf = ml_dtypes.bfloat16
    x = np.ascontiguousarray(inputs["x"], f).reshape(B, CS, N)
    l = np.ascontiguousarray(inputs["l"], f).reshape(B, CT, N)
    thwT = np.ascontiguousarray(inputs["theta_w"].T, f)   # [CS, CI]
    phwT = np.ascontiguousarray(inputs["phi_w"].T, f)
    gwT = np.asarray(inputs["g_w"].T, f)                  # [CS, CI]
    wwT = np.asarray(inputs["w_w"].T, f)                  # [CI, CT]
    pwb = np.concatenate([thwT[0:128], thwT[128:256],
                          phwT[0:128], phwT[128:256],
                          gwT[0:128], gwT[128:256], wwT], axis=1)
    pf = np.zeros((128, 10), f)
    pf[:, 0] = np.asarray(inputs["theta_b"], f)
    pf[:, 1] = np.asarray(inputs["phi_b"], f)
    for cth in range(2):
        hsl = slice(cth * 128, (cth + 1) * 128)
        pf[:, 2 + cth] = np.asarray(inputs["w_b"], f)[hsl]
        pf[:, 4 + cth] = np.asarray(inputs["bn_gamma"], f)[hsl]
        pf[:, 6 + cth] = np.asarray(inputs["bn_beta"], f)[hsl]
    shared = {
        "pwb": np.ascontiguousarray(pwb).astype(bf),
        "pf": pf,
        "g_b_bf": np.asarray(inputs["g_b"], f).reshape(1, CI).astype(bf),
    }
    return [{"x": x[i].astype(bf), "lres": l[i].astype(bf), **shared}
            for i in range(n_cores)]


def kernel(**inputs) -> np.ndarray:
    from concourse import bass_utils

    nc = _get_nc(N_CORES)
    in_maps = make_in_maps(inputs, N_CORES)
    res = bass_utils.run_bass_kernel_spmd(
        nc, in_maps, core_ids=list(range(N_CORES)))
    outs = [np.asarray(res.results[i]["out"], dtype=np.float32)
            for i in range(N_CORES)]
    return np.stack(outs, 0).reshape(B, CT, 64, 64)


if __name__ == "__main__":
    nc = _get_nc(1)
    print("build+compile OK")
